# revision 5
# baseline (speedup 1.0000x reference)
"""DeepGCN edge-update kernel for Trainium2 (8 NeuronCores, Bass/Tile).

Computes, for each edge e:
    h   = concat(x[src[e]], x[dst[e]])          # [2D]
    hn  = LayerNorm(h) * gamma + beta           # over 2D
    out = edge_attr[e] + relu(hn) @ W + b

Strategy (sharding_hint): shard edges across the 8 cores; replicate x and the
MLP params. The gather x[idx] uses the custom dma_gather Q7 instruction,
which takes int16 indices, so the host bucket-sorts each core's edges by
(src//32768, dst//32768) and issues per-bucket gathers with a base offset
into x. b is folded into edge_attr host-side; the output is un-permuted on
the host.

Self-contained: hardcodes the problem shapes (N=100000, E=600000, D=128).
"""

import math
import os

import numpy as np

N_NODES = 100000
N_EDGES = 600000
D = 128
TWO_D = 2 * D
N_CORES = 8
LN_EPS = 1e-5

BUCKET = 32768  # int16-addressable row range for dma_gather
N_BUCKETS = (N_NODES + BUCKET - 1) // BUCKET  # 4
P = 128
MAX_GATHER = 1024  # max num_idxs per dma_gather instruction (HW ring limit)
N_QUEUES = 4

# stash of the last BassKernelResults for test harnesses
last_results = None

_kernel_cache = {}


# ----------------------------------------------------------------------------
# host-side plan
# ----------------------------------------------------------------------------


def _build_plan(edge_index):
    """Bucket-sort each core's edges; return per-core permutations plus the
    shared (static) supertile plan.

    Returns dict with:
      perm[c]      : int64 [EPC] positions into the core's edge slice, sorted
      group_sizes  : int [16] padded group sizes (shared across cores)
      EP           : padded per-core edge count (multiple of 128)
      chunks       : list of (j0, n, sb, db) static gather chunks
    """
    src = edge_index[0].astype(np.int64)
    dst = edge_index[1].astype(np.int64)
    EPC = N_EDGES // N_CORES

    perms = []
    counts = np.zeros((N_CORES, N_BUCKETS * N_BUCKETS), dtype=np.int64)
    keys = []
    for c in range(N_CORES):
        s = src[c * EPC : (c + 1) * EPC]
        d = dst[c * EPC : (c + 1) * EPC]
        key = (s // BUCKET) * N_BUCKETS + (d // BUCKET)
        perm = np.argsort(key, kind="stable")
        perms.append(perm)
        keys.append(key[perm])
        counts[c] = np.bincount(key, minlength=N_BUCKETS * N_BUCKETS)

    gmax = counts.max(axis=0)
    group_sizes = ((gmax + P - 1) // P * P).astype(np.int64)
    EP = int(group_sizes.sum())

    chunks = []
    j0 = 0
    for g in range(N_BUCKETS * N_BUCKETS):
        n = int(group_sizes[g])
        sb, db = g // N_BUCKETS, g % N_BUCKETS
        off = 0
        while off < n:
            take = min(MAX_GATHER, n - off)
            chunks.append((j0 + off, take, sb, db))
            off += take
        j0 += n
    assert j0 == EP

    return {
        "perms": perms,
        "keys": keys,
        "counts": counts,
        "group_sizes": group_sizes,
        "EP": EP,
        "chunks": chunks,
        "EPC": EPC,
    }


def _wrap_idx(idx16):
    """[EP] int16 -> [128, EP//16] tile (16-partition wrap, replicated 8x)."""
    ep = idx16.shape[0]
    w = idx16.reshape(ep // 16, 16).T  # [16, S]
    return np.ascontiguousarray(np.tile(w, (8, 1)))


def _prep_core_inputs(plan, c, edge_index, edge_attr_plus_b):
    """Build the per-core padded/sorted arrays."""
    EPC, EP = plan["EPC"], plan["EP"]
    src = edge_index[0, c * EPC : (c + 1) * EPC].astype(np.int64)
    dst = edge_index[1, c * EPC : (c + 1) * EPC].astype(np.int64)
    perm = plan["perms"][c]
    key_sorted = plan["keys"][c]
    counts = plan["counts"][c]
    gs = plan["group_sizes"]

    src_s = src[perm]
    dst_s = dst[perm]
    ea_s = edge_attr_plus_b[c * EPC : (c + 1) * EPC][perm]

    src16 = np.zeros(EP, dtype=np.int16)
    dst16 = np.zeros(EP, dtype=np.int16)
    ea_pad = np.zeros((EP, D), dtype=np.float32)
    # slot[j] = index into the core's (unsorted) edge slice, or -1 for pads
    slot = np.full(EP, -1, dtype=np.int64)

    out_off = 0
    in_off = 0
    for g in range(N_BUCKETS * N_BUCKETS):
        n = int(counts[g])
        gp = int(gs[g])
        sb, db = g // N_BUCKETS, g % N_BUCKETS
        sl = slice(in_off, in_off + n)
        ol = slice(out_off, out_off + n)
        assert (key_sorted[sl] == g).all()
        src16[ol] = (src_s[sl] - sb * BUCKET).astype(np.int16)
        dst16[ol] = (dst_s[sl] - db * BUCKET).astype(np.int16)
        ea_pad[ol] = ea_s[sl]
        slot[ol] = perm[in_off : in_off + n]
        in_off += n
        out_off += gp
    assert in_off == EPC and out_off == EP

    return {
        "src_idx": _wrap_idx(src16),
        "dst_idx": _wrap_idx(dst16),
        "ea": ea_pad,
        "slot": slot,
    }


# ----------------------------------------------------------------------------
# bass kernel
# ----------------------------------------------------------------------------


def _build_bass(EP, chunks, affine):
    import concourse.bacc as bacc
    import concourse.bass as bass
    import concourse.tile as tile
    from concourse import mybir
    from concourse.masks import make_identity

    S_ALL = EP // 16
    fp32 = mybir.dt.float32
    fp16 = mybir.dt.float16
    MAXT = MAX_GATHER // P

    nc = bacc.Bacc(num_swdge_queues=N_QUEUES)
    x_d = nc.dram_tensor("x", (N_NODES, D), fp32, kind="ExternalInput")
    sidx_d = nc.dram_tensor("src_idx", (P, S_ALL), mybir.dt.int16, kind="ExternalInput")
    didx_d = nc.dram_tensor("dst_idx", (P, S_ALL), mybir.dt.int16, kind="ExternalInput")
    ea_d = nc.dram_tensor("ea", (EP, D), fp32, kind="ExternalInput")
    w_d = nc.dram_tensor("W", (TWO_D, D), fp32, kind="ExternalInput")
    if affine:
        gam_d = nc.dram_tensor("gamma", (TWO_D,), fp32, kind="ExternalInput")
        bet_d = nc.dram_tensor("beta", (TWO_D,), fp32, kind="ExternalInput")
    out_d = nc.dram_tensor("out", (EP, D), fp32, kind="ExternalOutput")

    ea_v = ea_d[:, :].rearrange("(t p) d -> p t d", p=P)  # [128, EP/128, 128]
    out_v = out_d[:, :].rearrange("(t p) d -> p t d", p=P)

    with tile.TileContext(nc) as tc:
        with (
            tc.tile_pool(name="const", bufs=1) as const,
            tc.tile_pool(name="h", bufs=3) as hpool,
            tc.tile_pool(name="io", bufs=3) as iopool,
            tc.tile_pool(name="z", bufs=6) as zpool,
            tc.tile_pool(name="st", bufs=4) as spool,
            tc.tile_pool(name="tp", bufs=3, space="PSUM") as tpsum,
            tc.tile_pool(name="om", bufs=3, space="PSUM") as opsum,
        ):
            # constants
            idx_s = const.tile([P, S_ALL], mybir.dt.int16)
            nc.sync.dma_start(out=idx_s[:], in_=sidx_d[:, :])
            idx_t = const.tile([P, S_ALL], mybir.dt.int16)
            nc.sync.dma_start(out=idx_t[:], in_=didx_d[:, :])
            w32 = const.tile([P, 2, D], fp32)  # [f, half, j]
            nc.sync.dma_start(
                out=w32[:],
                in_=w_d[:, :].rearrange("(h f) j -> f h j", h=2),
            )
            w16 = const.tile([P, 2, D], fp16)
            nc.vector.tensor_copy(out=w16[:], in_=w32[:])
            ident = const.tile([P, P], fp16)
            make_identity(nc, ident[:])
            eps_t = const.tile([P, 1], fp32)
            nc.vector.memset(eps_t[:], LN_EPS)
            if affine:
                gb = const.tile([P, 2, 2], fp32)  # [f, half, {gamma,beta}]
                nc.sync.dma_start(
                    out=gb[:, :, 0:1],
                    in_=gam_d[:].rearrange("(h f) -> f h 1", h=2),
                )
                nc.sync.dma_start(
                    out=gb[:, :, 1:2],
                    in_=bet_d[:].rearrange("(h f) -> f h 1", h=2),
                )

            gq = 0
            for j0, n, sb, db in chunks:
                T = n // P
                t0 = j0 // P
                # [p, half, t, d]; gather needs ap[1:]-contiguous dst slices
                hb = hpool.tile([P, 2, MAXT, D], fp32, tag="h")
                nc.gpsimd.dma_gather(
                    out_ap=hb[:, 0, :T, :],
                    in_ap=x_d[sb * BUCKET :, :],
                    idxs_ap=idx_s[:, j0 // 16 : (j0 + n) // 16],
                    num_idxs=n,
                    num_idxs_reg=n,
                    elem_size=D,
                    queue_num=gq % N_QUEUES,
                )
                nc.gpsimd.dma_gather(
                    out_ap=hb[:, 1, :T, :],
                    in_ap=x_d[db * BUCKET :, :],
                    idxs_ap=idx_t[:, j0 // 16 : (j0 + n) // 16],
                    num_idxs=n,
                    num_idxs_reg=n,
                    elem_size=D,
                    queue_num=(gq + 1) % N_QUEUES,
                )
                gq += 2

                ea_t = iopool.tile([P, MAXT, D], fp32, tag="ea")
                nc.sync.dma_start(out=ea_t[:, :T, :], in_=ea_v[:, t0 : t0 + T, :])
                oa = iopool.tile([P, MAXT, D], fp32, tag="oa")

                # per-tile LN stats
                stats = spool.tile([P, MAXT, 12], fp32, tag="stats")
                mv = spool.tile([P, MAXT, 2], fp32, tag="mv")
                for t in range(T):
                    nc.vector.bn_stats(out=stats[:, t, 0:6], in_=hb[:, 0, t, :])
                    nc.vector.bn_stats(out=stats[:, t, 6:12], in_=hb[:, 1, t, :])
                    nc.vector.bn_aggr(out=mv[:, t, :], in_=stats[:, t, :])
                # batched: rstd = 1/sqrt(var+eps), nmr = -mu*rstd
                sd = spool.tile([P, MAXT], fp32, tag="sd")
                nc.scalar.activation(
                    out=sd[:, :T],
                    in_=mv[:, :T, 1],
                    func=mybir.ActivationFunctionType.Sqrt,
                    bias=eps_t[:],
                )
                rstd = spool.tile([P, MAXT], fp32, tag="rstd")
                nc.vector.reciprocal(out=rstd[:, :T], in_=sd[:, :T])
                nmr = spool.tile([P, MAXT], fp32, tag="nmr")
                nc.vector.tensor_tensor(
                    out=nmr[:, :T],
                    in0=mv[:, :T, 0],
                    in1=rstd[:, :T],
                    op=mybir.AluOpType.mult,
                )
                nc.scalar.mul(out=nmr[:, :T], in_=nmr[:, :T], mul=-1.0)

                for t in range(T):
                    om = opsum.tile([P, D], fp32, tag="om")
                    t16 = zpool.tile([P, 2, D], fp16, tag="t16")
                    if affine:
                        # normalize only; gamma/beta + relu post-transpose
                        nc.scalar.activation(
                            out=t16[:],
                            in_=hb[:, :, t, :],
                            func=mybir.ActivationFunctionType.Identity,
                            bias=nmr[:, t : t + 1],
                            scale=rstd[:, t : t + 1],
                        )
                    else:
                        # relu((h-mu)*rstd) fused in one ACT pass
                        nc.scalar.activation(
                            out=t16[:],
                            in_=hb[:, :, t, :],
                            func=mybir.ActivationFunctionType.Relu,
                            bias=nmr[:, t : t + 1],
                            scale=rstd[:, t : t + 1],
                        )
                    tp = tpsum.tile([P, 2 * D], fp16, tag="tp")
                    nc.tensor.transpose(out=tp[:, 0:D], in_=t16[:, 0, :], identity=ident[:])
                    nc.tensor.transpose(out=tp[:, D : 2 * D], in_=t16[:, 1, :], identity=ident[:])
                    r = zpool.tile([P, 2 * D], fp16, tag="r")
                    if affine:
                        ga = zpool.tile([P, 2 * D], fp16, tag="ga")
                        for half in (0, 1):
                            nc.vector.tensor_scalar(
                                out=ga[:, half * D : (half + 1) * D],
                                in0=tp[:, half * D : (half + 1) * D],
                                scalar1=gb[:, half, 0:1],
                                scalar2=gb[:, half, 1:2],
                                op0=mybir.AluOpType.mult,
                                op1=mybir.AluOpType.add,
                            )
                        nc.scalar.activation(
                            out=r[:], in_=ga[:],
                            func=mybir.ActivationFunctionType.Relu,
                        )
                    else:
                        nc.scalar.activation(
                            out=r[:], in_=tp[:],
                            func=mybir.ActivationFunctionType.Copy,
                        )
                    nc.tensor.matmul(
                        out=om[:],
                        lhsT=r[:, 0:D],
                        rhs=w16[:, 0, :],
                        start=True,
                        stop=False,
                    )
                    nc.tensor.matmul(
                        out=om[:],
                        lhsT=r[:, D : 2 * D],
                        rhs=w16[:, 1, :],
                        start=False,
                        stop=True,
                    )
                    nc.vector.tensor_tensor(
                        out=oa[:, t, :],
                        in0=om[:],
                        in1=ea_t[:, t, :],
                        op=mybir.AluOpType.add,
                    )
                nc.sync.dma_start(out=out_v[:, t0 : t0 + T, :], in_=oa[:, :T, :])

    # Each DMA semaphore may only ever be incremented from one SWDGE queue
    # (ucode shadow-sem invariant). Tile assigns DMASW lanes in scheduled
    # order, so re-derive queue_num from the assigned lane (lane % N_QUEUES).
    import re

    for blk in nc.m.functions[0].blocks:
        for inst in blk.instructions:
            if isinstance(inst, mybir.InstDMAGatherAnt):
                name = inst.sync_info.on_update[0].ant_name
                m = re.match(r"DMASW(\d+)_", name)
                assert m, name
                inst.queue_num = int(m.group(1)) % N_QUEUES

    nc.compile()
    return nc


# ----------------------------------------------------------------------------
# entry point
# ----------------------------------------------------------------------------


def kernel(x, edge_index, edge_attr, ln_gamma, ln_beta, W, b):
    global last_results
    from concourse import bass_utils

    x = np.ascontiguousarray(np.asarray(x, dtype=np.float32))
    edge_attr = np.asarray(edge_attr, dtype=np.float32)
    W_f = np.ascontiguousarray(np.asarray(W, dtype=np.float32))
    b_f = np.asarray(b, dtype=np.float32)
    gamma = np.asarray(ln_gamma, dtype=np.float32)
    beta = np.asarray(ln_beta, dtype=np.float32)
    ei = np.asarray(edge_index)

    affine = not (np.all(gamma == 1.0) and np.all(beta == 0.0))

    plan = _build_plan(ei)
    EP = plan["EP"]

    key = (EP, tuple(plan["chunks"]), affine)
    if key not in _kernel_cache:
        _kernel_cache.clear()
        _kernel_cache[key] = _build_bass(EP, plan["chunks"], affine)
    nc = _kernel_cache[key]

    ea_plus_b = edge_attr + b_f[None, :]

    in_maps = []
    slots = []
    for c in range(N_CORES):
        ci = _prep_core_inputs(plan, c, ei, ea_plus_b)
        m = {
            "x": x,
            "src_idx": ci["src_idx"],
            "dst_idx": ci["dst_idx"],
            "ea": ci["ea"],
            "W": W_f,
        }
        if affine:
            m["gamma"] = gamma
            m["beta"] = beta
        in_maps.append(m)
        slots.append(ci["slot"])

    res = bass_utils.run_bass_kernel_spmd(nc, in_maps, core_ids=list(range(N_CORES)))
    last_results = res

    out = np.empty((N_EDGES, D), dtype=np.float32)
    EPC = plan["EPC"]
    for c in range(N_CORES):
        oc = res.results[c]["out"]
        sl = slots[c]
        valid = sl >= 0
        out[c * EPC + sl[valid]] = oc[valid]
    return out


# revision 6
# speedup vs baseline: 1.1414x; 1.1414x over previous
"""DeepGCN edge-update kernel for Trainium2 (8 NeuronCores, Bass/Tile).

Computes, for each edge e:
    h   = concat(x[src[e]], x[dst[e]])          # [2D]
    hn  = LayerNorm(h) * gamma + beta           # over 2D
    out = edge_attr[e] + relu(hn) @ W + b

Strategy (sharding_hint): shard edges across the 8 cores; replicate x and the
MLP params. The gather x[idx] uses the custom dma_gather Q7 instruction,
which takes int16 indices, so the host bucket-sorts each core's edges by
(src//32768, dst//32768) and issues per-bucket gathers with a base offset
into x. b is folded into edge_attr host-side; the output is un-permuted on
the host.

Self-contained: hardcodes the problem shapes (N=100000, E=600000, D=128).
"""

import math
import os

import numpy as np

N_NODES = 100000
N_EDGES = 600000
D = 128
TWO_D = 2 * D
N_CORES = 8
LN_EPS = 1e-5

BUCKET = 32768  # int16-addressable row range for dma_gather
N_BUCKETS = (N_NODES + BUCKET - 1) // BUCKET  # 4
P = 128
MAX_GATHER = 1024  # max num_idxs per dma_gather instruction (HW ring limit)
N_QUEUES = 4

# stash of the last BassKernelResults for test harnesses
last_results = None

_kernel_cache = {}


# ----------------------------------------------------------------------------
# host-side plan
# ----------------------------------------------------------------------------


def _build_plan(edge_index):
    """Bucket-sort each core's edges; return per-core permutations plus the
    shared (static) supertile plan.

    Returns dict with:
      perm[c]      : int64 [EPC] positions into the core's edge slice, sorted
      group_sizes  : int [16] padded group sizes (shared across cores)
      EP           : padded per-core edge count (multiple of 128)
      chunks       : list of (j0, n, sb, db) static gather chunks
    """
    src = edge_index[0].astype(np.int64)
    dst = edge_index[1].astype(np.int64)
    EPC = N_EDGES // N_CORES

    perms = []
    counts = np.zeros((N_CORES, N_BUCKETS * N_BUCKETS), dtype=np.int64)
    keys = []
    for c in range(N_CORES):
        s = src[c * EPC : (c + 1) * EPC]
        d = dst[c * EPC : (c + 1) * EPC]
        key = (s // BUCKET) * N_BUCKETS + (d // BUCKET)
        perm = np.argsort(key, kind="stable")
        perms.append(perm)
        keys.append(key[perm])
        counts[c] = np.bincount(key, minlength=N_BUCKETS * N_BUCKETS)

    gmax = counts.max(axis=0)
    group_sizes = ((gmax + P - 1) // P * P).astype(np.int64)
    EP = int(group_sizes.sum())

    chunks = []
    j0 = 0
    for g in range(N_BUCKETS * N_BUCKETS):
        n = int(group_sizes[g])
        sb, db = g // N_BUCKETS, g % N_BUCKETS
        off = 0
        while off < n:
            take = min(MAX_GATHER, n - off)
            chunks.append((j0 + off, take, sb, db))
            off += take
        j0 += n
    assert j0 == EP

    return {
        "perms": perms,
        "keys": keys,
        "counts": counts,
        "group_sizes": group_sizes,
        "EP": EP,
        "chunks": chunks,
        "EPC": EPC,
    }


def _wrap_idx(idx16):
    """[EP] int16 -> [128, EP//16] tile (16-partition wrap, replicated 8x)."""
    ep = idx16.shape[0]
    w = idx16.reshape(ep // 16, 16).T  # [16, S]
    return np.ascontiguousarray(np.tile(w, (8, 1)))


def _prep_core_inputs(plan, c, edge_index, edge_attr_plus_b):
    """Build the per-core padded/sorted arrays."""
    EPC, EP = plan["EPC"], plan["EP"]
    src = edge_index[0, c * EPC : (c + 1) * EPC].astype(np.int64)
    dst = edge_index[1, c * EPC : (c + 1) * EPC].astype(np.int64)
    perm = plan["perms"][c]
    key_sorted = plan["keys"][c]
    counts = plan["counts"][c]
    gs = plan["group_sizes"]

    src_s = src[perm]
    dst_s = dst[perm]
    ea_s = edge_attr_plus_b[c * EPC : (c + 1) * EPC][perm]

    src16 = np.zeros(EP, dtype=np.int16)
    dst16 = np.zeros(EP, dtype=np.int16)
    ea_pad = np.zeros((EP, D), dtype=np.float32)
    # slot[j] = index into the core's (unsorted) edge slice, or -1 for pads
    slot = np.full(EP, -1, dtype=np.int64)

    out_off = 0
    in_off = 0
    for g in range(N_BUCKETS * N_BUCKETS):
        n = int(counts[g])
        gp = int(gs[g])
        sb, db = g // N_BUCKETS, g % N_BUCKETS
        sl = slice(in_off, in_off + n)
        ol = slice(out_off, out_off + n)
        assert (key_sorted[sl] == g).all()
        src16[ol] = (src_s[sl] - sb * BUCKET).astype(np.int16)
        dst16[ol] = (dst_s[sl] - db * BUCKET).astype(np.int16)
        ea_pad[ol] = ea_s[sl]
        slot[ol] = perm[in_off : in_off + n]
        in_off += n
        out_off += gp
    assert in_off == EPC and out_off == EP

    ea_t = np.ascontiguousarray(ea_pad.reshape(EP // P, P, D).transpose(1, 0, 2))
    return {
        "src_idx": _wrap_idx(src16),
        "dst_idx": _wrap_idx(dst16),
        "ea": ea_t,
        "slot": slot,
    }


# ----------------------------------------------------------------------------
# bass kernel
# ----------------------------------------------------------------------------


def _build_bass(EP, chunks, affine):
    import concourse.bacc as bacc
    import concourse.bass as bass
    import concourse.tile as tile
    from concourse import mybir
    from concourse.masks import make_identity

    S_ALL = EP // 16
    fp32 = mybir.dt.float32
    fp16 = mybir.dt.float16
    MAXT = MAX_GATHER // P

    nc = bacc.Bacc(num_swdge_queues=N_QUEUES, dynamic_dma_scratch_size=49152)
    x_d = nc.dram_tensor("x", (N_NODES, D), fp32, kind="ExternalInput")
    sidx_d = nc.dram_tensor("src_idx", (P, S_ALL), mybir.dt.int16, kind="ExternalInput")
    didx_d = nc.dram_tensor("dst_idx", (P, S_ALL), mybir.dt.int16, kind="ExternalInput")
    ea_d = nc.dram_tensor("ea", (P, EP // P, D), fp32, kind="ExternalInput")
    w_d = nc.dram_tensor("W", (TWO_D, D), fp32, kind="ExternalInput")
    if affine:
        gam_d = nc.dram_tensor("gamma", (TWO_D,), fp32, kind="ExternalInput")
        bet_d = nc.dram_tensor("beta", (TWO_D,), fp32, kind="ExternalInput")
    out_d = nc.dram_tensor("out", (P, EP // P, D), fp32, kind="ExternalOutput")

    ea_v = ea_d[:, :, :]  # [128, EP/128, 128] tile-transposed on host
    out_v = out_d[:, :, :]

    with tile.TileContext(nc) as tc:
        with (
            tc.tile_pool(name="const", bufs=1) as const,
            tc.tile_pool(name="h", bufs=5) as hpool,
            tc.tile_pool(name="io", bufs=3) as iopool,
            tc.tile_pool(name="z", bufs=6) as zpool,
            tc.tile_pool(name="st", bufs=4) as spool,
            tc.tile_pool(name="tp", bufs=3, space="PSUM") as tpsum,
            tc.tile_pool(name="om", bufs=3, space="PSUM") as opsum,
        ):
            # constants
            idx_s = const.tile([P, S_ALL], mybir.dt.int16)
            nc.sync.dma_start(out=idx_s[:], in_=sidx_d[:, :])
            idx_t = const.tile([P, S_ALL], mybir.dt.int16)
            nc.sync.dma_start(out=idx_t[:], in_=didx_d[:, :])
            w32 = const.tile([P, 2, D], fp32)  # [f, half, j]
            nc.sync.dma_start(
                out=w32[:],
                in_=w_d[:, :].rearrange("(h f) j -> f h j", h=2),
            )
            w16 = const.tile([P, 2, D], fp16)
            nc.vector.tensor_copy(out=w16[:], in_=w32[:])
            ident = const.tile([P, P], fp16)
            make_identity(nc, ident[:])
            eps_t = const.tile([P, 1], fp32)
            nc.vector.memset(eps_t[:], LN_EPS)
            if affine:
                gb = const.tile([P, 2, 2], fp32)  # [f, half, {gamma,beta}]
                nc.sync.dma_start(
                    out=gb[:, :, 0:1],
                    in_=gam_d[:].rearrange("(h f) -> f h 1", h=2),
                )
                nc.sync.dma_start(
                    out=gb[:, :, 1:2],
                    in_=bet_d[:].rearrange("(h f) -> f h 1", h=2),
                )

            gq = 0
            for j0, n, sb, db in chunks:
                T = n // P
                t0 = j0 // P
                # [p, half, t, d]; gather needs ap[1:]-contiguous dst slices
                hb = hpool.tile([P, 2, MAXT, D], fp32, tag="h")
                nc.gpsimd.dma_gather(
                    out_ap=hb[:, 0, :T, :],
                    in_ap=x_d[sb * BUCKET :, :],
                    idxs_ap=idx_s[:, j0 // 16 : (j0 + n) // 16],
                    num_idxs=n,
                    num_idxs_reg=n,
                    elem_size=D,
                    queue_num=gq % N_QUEUES,
                )
                nc.gpsimd.dma_gather(
                    out_ap=hb[:, 1, :T, :],
                    in_ap=x_d[db * BUCKET :, :],
                    idxs_ap=idx_t[:, j0 // 16 : (j0 + n) // 16],
                    num_idxs=n,
                    num_idxs_reg=n,
                    elem_size=D,
                    queue_num=(gq + 1) % N_QUEUES,
                )
                gq += 2

                ea_t = iopool.tile([P, MAXT, D], fp32, tag="ea")
                nc.sync.dma_start(out=ea_t[:, :T, :], in_=ea_v[:, t0 : t0 + T, :])
                oa = iopool.tile([P, MAXT, D], fp32, tag="oa")

                # per-tile LN stats
                stats = spool.tile([P, MAXT, 12], fp32, tag="stats")
                mv = spool.tile([P, MAXT, 2], fp32, tag="mv")
                for t in range(T):
                    nc.vector.bn_stats(out=stats[:, t, 0:6], in_=hb[:, 0, t, :])
                    nc.vector.bn_stats(out=stats[:, t, 6:12], in_=hb[:, 1, t, :])
                    nc.vector.bn_aggr(out=mv[:, t, :], in_=stats[:, t, :])
                # batched: rstd = 1/sqrt(var+eps), nmr = -mu*rstd
                sd = spool.tile([P, MAXT], fp32, tag="sd")
                nc.scalar.activation(
                    out=sd[:, :T],
                    in_=mv[:, :T, 1],
                    func=mybir.ActivationFunctionType.Sqrt,
                    bias=eps_t[:],
                )
                rstd = spool.tile([P, MAXT], fp32, tag="rstd")
                nc.vector.reciprocal(out=rstd[:, :T], in_=sd[:, :T])
                nmr = spool.tile([P, MAXT], fp32, tag="nmr")
                nc.vector.tensor_tensor(
                    out=nmr[:, :T],
                    in0=mv[:, :T, 0],
                    in1=rstd[:, :T],
                    op=mybir.AluOpType.mult,
                )
                nc.scalar.mul(out=nmr[:, :T], in_=nmr[:, :T], mul=-1.0)

                for t in range(T):
                    om = opsum.tile([P, D], fp32, tag="om")
                    t16 = zpool.tile([P, 2, D], fp16, tag="t16")
                    if affine:
                        # normalize only; gamma/beta + relu post-transpose
                        nc.scalar.activation(
                            out=t16[:],
                            in_=hb[:, :, t, :],
                            func=mybir.ActivationFunctionType.Identity,
                            bias=nmr[:, t : t + 1],
                            scale=rstd[:, t : t + 1],
                        )
                    else:
                        # relu((h-mu)*rstd) fused in one ACT pass
                        nc.scalar.activation(
                            out=t16[:],
                            in_=hb[:, :, t, :],
                            func=mybir.ActivationFunctionType.Relu,
                            bias=nmr[:, t : t + 1],
                            scale=rstd[:, t : t + 1],
                        )
                    tp = tpsum.tile([P, 2 * D], fp16, tag="tp")
                    nc.tensor.transpose(out=tp[:, 0:D], in_=t16[:, 0, :], identity=ident[:])
                    nc.tensor.transpose(out=tp[:, D : 2 * D], in_=t16[:, 1, :], identity=ident[:])
                    r = zpool.tile([P, 2 * D], fp16, tag="r")
                    if affine:
                        ga = zpool.tile([P, 2 * D], fp16, tag="ga")
                        for half in (0, 1):
                            nc.vector.tensor_scalar(
                                out=ga[:, half * D : (half + 1) * D],
                                in0=tp[:, half * D : (half + 1) * D],
                                scalar1=gb[:, half, 0:1],
                                scalar2=gb[:, half, 1:2],
                                op0=mybir.AluOpType.mult,
                                op1=mybir.AluOpType.add,
                            )
                        nc.scalar.activation(
                            out=r[:], in_=ga[:],
                            func=mybir.ActivationFunctionType.Relu,
                        )
                    else:
                        nc.scalar.activation(
                            out=r[:], in_=tp[:],
                            func=mybir.ActivationFunctionType.Copy,
                        )
                    nc.tensor.matmul(
                        out=om[:],
                        lhsT=r[:, 0:D],
                        rhs=w16[:, 0, :],
                        start=True,
                        stop=False,
                    )
                    nc.tensor.matmul(
                        out=om[:],
                        lhsT=r[:, D : 2 * D],
                        rhs=w16[:, 1, :],
                        start=False,
                        stop=True,
                    )
                    nc.vector.tensor_tensor(
                        out=oa[:, t, :],
                        in0=om[:],
                        in1=ea_t[:, t, :],
                        op=mybir.AluOpType.add,
                    )
                nc.sync.dma_start(out=out_v[:, t0 : t0 + T, :], in_=oa[:, :T, :])

    # Each DMA semaphore may only ever be incremented from one SWDGE queue
    # (ucode shadow-sem invariant). Tile assigns DMASW lanes in scheduled
    # order, so re-derive queue_num from the assigned lane (lane % N_QUEUES).
    import re

    for blk in nc.m.functions[0].blocks:
        for inst in blk.instructions:
            if isinstance(inst, mybir.InstDMAGatherAnt):
                name = inst.sync_info.on_update[0].ant_name
                m = re.match(r"DMASW(\d+)_", name)
                assert m, name
                inst.queue_num = int(m.group(1)) % N_QUEUES

    nc.compile()
    return nc


# ----------------------------------------------------------------------------
# entry point
# ----------------------------------------------------------------------------


def kernel(x, edge_index, edge_attr, ln_gamma, ln_beta, W, b):
    global last_results
    from concourse import bass_utils

    x = np.ascontiguousarray(np.asarray(x, dtype=np.float32))
    edge_attr = np.asarray(edge_attr, dtype=np.float32)
    W_f = np.ascontiguousarray(np.asarray(W, dtype=np.float32))
    b_f = np.asarray(b, dtype=np.float32)
    gamma = np.asarray(ln_gamma, dtype=np.float32)
    beta = np.asarray(ln_beta, dtype=np.float32)
    ei = np.asarray(edge_index)

    affine = not (np.all(gamma == 1.0) and np.all(beta == 0.0))

    plan = _build_plan(ei)
    EP = plan["EP"]

    key = (EP, tuple(plan["chunks"]), affine)
    if key not in _kernel_cache:
        _kernel_cache.clear()
        _kernel_cache[key] = _build_bass(EP, plan["chunks"], affine)
    nc = _kernel_cache[key]

    ea_plus_b = edge_attr + b_f[None, :]

    in_maps = []
    slots = []
    for c in range(N_CORES):
        ci = _prep_core_inputs(plan, c, ei, ea_plus_b)
        m = {
            "x": x,
            "src_idx": ci["src_idx"],
            "dst_idx": ci["dst_idx"],
            "ea": ci["ea"],
            "W": W_f,
        }
        if affine:
            m["gamma"] = gamma
            m["beta"] = beta
        in_maps.append(m)
        slots.append(ci["slot"])

    res = bass_utils.run_bass_kernel_spmd(nc, in_maps, core_ids=list(range(N_CORES)))
    last_results = res

    out = np.empty((N_EDGES, D), dtype=np.float32)
    EPC = plan["EPC"]
    for c in range(N_CORES):
        oc = res.results[c]["out"].transpose(1, 0, 2).reshape(-1, D)
        sl = slots[c]
        valid = sl >= 0
        out[c * EPC + sl[valid]] = oc[valid]
    return out


# revision 7
# speedup vs baseline: 1.8896x; 1.6556x over previous
"""DeepGCN edge-update kernel for Trainium2 (8 NeuronCores, Bass/Tile).

Computes, for each edge e:
    h   = concat(x[src[e]], x[dst[e]])          # [2D]
    hn  = LayerNorm(h) * gamma + beta           # over 2D
    out = edge_attr[e] + relu(hn) @ W + b

Strategy (sharding_hint): shard edges across the 8 cores; replicate x and the
MLP params. The gather x[idx] uses the custom dma_gather Q7 instruction,
which takes int16 indices, so the host bucket-sorts each core's edges by
(src//32768, dst//32768) and issues per-bucket gathers with a base offset
into x. b is folded into edge_attr host-side; the output is un-permuted on
the host.

Self-contained: hardcodes the problem shapes (N=100000, E=600000, D=128).
"""

import math
import os

import numpy as np

N_NODES = 100000
N_EDGES = 600000
D = 128
TWO_D = 2 * D
N_CORES = 8
LN_EPS = 1e-5

BUCKET = 32768  # int16-addressable row range for dma_gather
N_BUCKETS = (N_NODES + BUCKET - 1) // BUCKET  # 4
P = 128
MAX_GATHER = 1024  # max num_idxs per dma_gather instruction (HW ring limit)
GRP = 4  # tiles per wide matmul group (N = GRP*128)
N_QUEUES = 4

# stash of the last BassKernelResults for test harnesses
last_results = None

_kernel_cache = {}


# ----------------------------------------------------------------------------
# host-side plan
# ----------------------------------------------------------------------------


def _build_plan(edge_index):
    """Bucket-sort each core's edges; return per-core permutations plus the
    shared (static) supertile plan.

    Returns dict with:
      perm[c]      : int64 [EPC] positions into the core's edge slice, sorted
      group_sizes  : int [16] padded group sizes (shared across cores)
      EP           : padded per-core edge count (multiple of 128)
      chunks       : list of (j0, n, sb, db) static gather chunks
    """
    src = edge_index[0].astype(np.int64)
    dst = edge_index[1].astype(np.int64)
    EPC = N_EDGES // N_CORES

    perms = []
    counts = np.zeros((N_CORES, N_BUCKETS * N_BUCKETS), dtype=np.int64)
    keys = []
    for c in range(N_CORES):
        s = src[c * EPC : (c + 1) * EPC]
        d = dst[c * EPC : (c + 1) * EPC]
        key = (s // BUCKET) * N_BUCKETS + (d // BUCKET)
        perm = np.argsort(key, kind="stable")
        perms.append(perm)
        keys.append(key[perm])
        counts[c] = np.bincount(key, minlength=N_BUCKETS * N_BUCKETS)

    gmax = counts.max(axis=0)
    group_sizes = ((gmax + P - 1) // P * P).astype(np.int64)
    EP = int(group_sizes.sum())

    chunks = []
    j0 = 0
    for g in range(N_BUCKETS * N_BUCKETS):
        n = int(group_sizes[g])
        sb, db = g // N_BUCKETS, g % N_BUCKETS
        off = 0
        while off < n:
            take = min(MAX_GATHER, n - off)
            chunks.append((j0 + off, take, sb, db))
            off += take
        j0 += n
    assert j0 == EP

    return {
        "perms": perms,
        "keys": keys,
        "counts": counts,
        "group_sizes": group_sizes,
        "EP": EP,
        "chunks": chunks,
        "EPC": EPC,
    }


def _wrap_idx(idx16):
    """[EP] int16 -> [128, EP//16] tile (16-partition wrap, replicated 8x)."""
    ep = idx16.shape[0]
    w = idx16.reshape(ep // 16, 16).T  # [16, S]
    return np.ascontiguousarray(np.tile(w, (8, 1)))


def _prep_core_inputs(plan, c, edge_index, edge_attr_plus_b):
    """Build the per-core padded/sorted arrays."""
    EPC, EP = plan["EPC"], plan["EP"]
    src = edge_index[0, c * EPC : (c + 1) * EPC].astype(np.int64)
    dst = edge_index[1, c * EPC : (c + 1) * EPC].astype(np.int64)
    perm = plan["perms"][c]
    key_sorted = plan["keys"][c]
    counts = plan["counts"][c]
    gs = plan["group_sizes"]

    src_s = src[perm]
    dst_s = dst[perm]
    ea_s = edge_attr_plus_b[c * EPC : (c + 1) * EPC][perm]

    src16 = np.zeros(EP, dtype=np.int16)
    dst16 = np.zeros(EP, dtype=np.int16)
    ea_pad = np.zeros((EP, D), dtype=np.float32)
    # slot[j] = index into the core's (unsorted) edge slice, or -1 for pads
    slot = np.full(EP, -1, dtype=np.int64)

    out_off = 0
    in_off = 0
    for g in range(N_BUCKETS * N_BUCKETS):
        n = int(counts[g])
        gp = int(gs[g])
        sb, db = g // N_BUCKETS, g % N_BUCKETS
        sl = slice(in_off, in_off + n)
        ol = slice(out_off, out_off + n)
        assert (key_sorted[sl] == g).all()
        src16[ol] = (src_s[sl] - sb * BUCKET).astype(np.int16)
        dst16[ol] = (dst_s[sl] - db * BUCKET).astype(np.int16)
        ea_pad[ol] = ea_s[sl]
        slot[ol] = perm[in_off : in_off + n]
        in_off += n
        out_off += gp
    assert in_off == EPC and out_off == EP

    ea_t = np.ascontiguousarray(ea_pad.T)  # [D, EP] feature-major
    return {
        "src_idx": _wrap_idx(src16),
        "dst_idx": _wrap_idx(dst16),
        "ea": ea_t,
        "slot": slot,
    }


# ----------------------------------------------------------------------------
# bass kernel
# ----------------------------------------------------------------------------


def _build_bass(EP, chunks, affine):
    import concourse.bacc as bacc
    import concourse.bass as bass
    import concourse.tile as tile
    from concourse import mybir
    from concourse.masks import make_identity

    S_ALL = EP // 16
    fp32 = mybir.dt.float32
    fp16 = mybir.dt.float16
    MAXT = MAX_GATHER // P

    nc = bacc.Bacc(num_swdge_queues=N_QUEUES, dynamic_dma_scratch_size=49152)
    x_d = nc.dram_tensor("x", (N_NODES, D), fp32, kind="ExternalInput")
    sidx_d = nc.dram_tensor("src_idx", (P, S_ALL), mybir.dt.int16, kind="ExternalInput")
    didx_d = nc.dram_tensor("dst_idx", (P, S_ALL), mybir.dt.int16, kind="ExternalInput")
    ea_d = nc.dram_tensor("ea", (D, EP), fp32, kind="ExternalInput")
    w_d = nc.dram_tensor("W", (TWO_D, D), fp32, kind="ExternalInput")
    if affine:
        gam_d = nc.dram_tensor("gamma", (TWO_D,), fp32, kind="ExternalInput")
        bet_d = nc.dram_tensor("beta", (TWO_D,), fp32, kind="ExternalInput")
    out_d = nc.dram_tensor("out", (D, EP), fp32, kind="ExternalOutput")

    ea_v = ea_d[:, :]  # [D, EP] feature-major (host pre-transposed)
    out_v = out_d[:, :]

    with tile.TileContext(nc) as tc:
        with (
            tc.tile_pool(name="const", bufs=1) as const,
            tc.tile_pool(name="h", bufs=5) as hpool,
            tc.tile_pool(name="io", bufs=3) as iopool,
            tc.tile_pool(name="z", bufs=6) as zpool,
            tc.tile_pool(name="st", bufs=4) as spool,
            tc.tile_pool(name="tp", bufs=3, space="PSUM") as tpsum,
            tc.tile_pool(name="om", bufs=3, space="PSUM") as opsum,
        ):
            # constants
            idx_s = const.tile([P, S_ALL], mybir.dt.int16)
            nc.sync.dma_start(out=idx_s[:], in_=sidx_d[:, :])
            idx_t = const.tile([P, S_ALL], mybir.dt.int16)
            nc.sync.dma_start(out=idx_t[:], in_=didx_d[:, :])
            w32 = const.tile([P, 2, D], fp32)  # [f, half, j]
            nc.sync.dma_start(
                out=w32[:],
                in_=w_d[:, :].rearrange("(h f) j -> f h j", h=2),
            )
            w16 = const.tile([P, 2, D], fp16)
            nc.vector.tensor_copy(out=w16[:], in_=w32[:])
            ident = const.tile([P, P], fp16)
            make_identity(nc, ident[:])
            eps_t = const.tile([P, 1], fp32)
            nc.vector.memset(eps_t[:], LN_EPS)
            if affine:
                gb = const.tile([P, 2, 2], fp32)  # [f, half, {gamma,beta}]
                nc.sync.dma_start(
                    out=gb[:, :, 0:1],
                    in_=gam_d[:].rearrange("(h f) -> f h 1", h=2),
                )
                nc.sync.dma_start(
                    out=gb[:, :, 1:2],
                    in_=bet_d[:].rearrange("(h f) -> f h 1", h=2),
                )

            gq = 0
            for j0, n, sb, db in chunks:
                T = n // P
                t0 = j0 // P
                # [p, half, t, d]; gather needs ap[1:]-contiguous dst slices
                hb = hpool.tile([P, 2, MAXT, D], fp32, tag="h")
                nc.gpsimd.dma_gather(
                    out_ap=hb[:, 0, :T, :],
                    in_ap=x_d[sb * BUCKET :, :],
                    idxs_ap=idx_s[:, j0 // 16 : (j0 + n) // 16],
                    num_idxs=n,
                    num_idxs_reg=n,
                    elem_size=D,
                    queue_num=gq % N_QUEUES,
                )
                nc.gpsimd.dma_gather(
                    out_ap=hb[:, 1, :T, :],
                    in_ap=x_d[db * BUCKET :, :],
                    idxs_ap=idx_t[:, j0 // 16 : (j0 + n) // 16],
                    num_idxs=n,
                    num_idxs_reg=n,
                    elem_size=D,
                    queue_num=(gq + 1) % N_QUEUES,
                )
                gq += 2

                ea_t = iopool.tile([P, MAX_GATHER], fp32, tag="ea")
                nc.sync.dma_start(out=ea_t[:, :n], in_=ea_v[:, j0 : j0 + n])
                oa = iopool.tile([P, MAX_GATHER], fp32, tag="oa")

                # per-tile LN stats
                stats = spool.tile([P, MAXT, 12], fp32, tag="stats")
                mv = spool.tile([P, MAXT, 2], fp32, tag="mv")
                for t in range(T):
                    nc.vector.bn_stats(out=stats[:, t, 0:6], in_=hb[:, 0, t, :])
                    nc.vector.bn_stats(out=stats[:, t, 6:12], in_=hb[:, 1, t, :])
                    nc.vector.bn_aggr(out=mv[:, t, :], in_=stats[:, t, :])
                # batched: rstd = 1/sqrt(var+eps), nmr = -mu*rstd
                sd = spool.tile([P, MAXT], fp32, tag="sd")
                nc.scalar.activation(
                    out=sd[:, :T],
                    in_=mv[:, :T, 1],
                    func=mybir.ActivationFunctionType.Sqrt,
                    bias=eps_t[:],
                )
                rstd = spool.tile([P, MAXT], fp32, tag="rstd")
                nc.vector.reciprocal(out=rstd[:, :T], in_=sd[:, :T])
                nmr = spool.tile([P, MAXT], fp32, tag="nmr")
                nc.vector.tensor_tensor(
                    out=nmr[:, :T],
                    in0=mv[:, :T, 0],
                    in1=rstd[:, :T],
                    op=mybir.AluOpType.mult,
                )
                nc.scalar.mul(out=nmr[:, :T], in_=nmr[:, :T], mul=-1.0)

                for g0 in range(0, T, GRP):
                    g = min(GRP, T - g0)
                    tpg = tpsum.tile([P, 2, GRP * P], fp16, tag="tp")
                    for ti in range(g):
                        t = g0 + ti
                        t16 = zpool.tile([P, 2, D], fp16, tag="t16")
                        if affine:
                            nc.scalar.activation(
                                out=t16[:],
                                in_=hb[:, :, t, :],
                                func=mybir.ActivationFunctionType.Identity,
                                bias=nmr[:, t : t + 1],
                                scale=rstd[:, t : t + 1],
                            )
                        else:
                            nc.scalar.activation(
                                out=t16[:],
                                in_=hb[:, :, t, :],
                                func=mybir.ActivationFunctionType.Relu,
                                bias=nmr[:, t : t + 1],
                                scale=rstd[:, t : t + 1],
                            )
                        nc.tensor.transpose(
                            out=tpg[:, 0, ti * P : (ti + 1) * P],
                            in_=t16[:, 0, :],
                            identity=ident[:],
                        )
                        nc.tensor.transpose(
                            out=tpg[:, 1, ti * P : (ti + 1) * P],
                            in_=t16[:, 1, :],
                            identity=ident[:],
                        )
                    r = zpool.tile([P, 2, GRP * P], fp16, tag="r")
                    if affine:
                        for half in (0, 1):
                            nc.vector.tensor_scalar(
                                out=r[:, half, : g * P],
                                in0=tpg[:, half, : g * P],
                                scalar1=gb[:, half, 0:1],
                                scalar2=gb[:, half, 1:2],
                                op0=mybir.AluOpType.mult,
                                op1=mybir.AluOpType.add,
                            )
                        nc.scalar.activation(
                            out=r[:, :, : g * P],
                            in_=r[:, :, : g * P],
                            func=mybir.ActivationFunctionType.Relu,
                        )
                    else:
                        for half in (0, 1):
                            nc.scalar.activation(
                                out=r[:, half, : g * P],
                                in_=tpg[:, half, : g * P],
                                func=mybir.ActivationFunctionType.Copy,
                            )
                    om = opsum.tile([P, GRP * P], fp32, tag="om")
                    nc.tensor.matmul(
                        out=om[:, : g * P],
                        lhsT=w16[:, 0, :],
                        rhs=r[:, 0, : g * P],
                        start=True,
                        stop=False,
                    )
                    nc.tensor.matmul(
                        out=om[:, : g * P],
                        lhsT=w16[:, 1, :],
                        rhs=r[:, 1, : g * P],
                        start=False,
                        stop=True,
                    )
                    nc.vector.tensor_tensor(
                        out=oa[:, g0 * P : (g0 + g) * P],
                        in0=om[:, : g * P],
                        in1=ea_t[:, g0 * P : (g0 + g) * P],
                        op=mybir.AluOpType.add,
                    )
                nc.sync.dma_start(out=out_v[:, j0 : j0 + n], in_=oa[:, :n])

    # Each DMA semaphore may only ever be incremented from one SWDGE queue
    # (ucode shadow-sem invariant). Tile assigns DMASW lanes in scheduled
    # order, so re-derive queue_num from the assigned lane (lane % N_QUEUES).
    import re

    for blk in nc.m.functions[0].blocks:
        for inst in blk.instructions:
            if isinstance(inst, mybir.InstDMAGatherAnt):
                name = inst.sync_info.on_update[0].ant_name
                m = re.match(r"DMASW(\d+)_", name)
                assert m, name
                inst.queue_num = int(m.group(1)) % N_QUEUES

    nc.compile()
    return nc


# ----------------------------------------------------------------------------
# entry point
# ----------------------------------------------------------------------------


def kernel(x, edge_index, edge_attr, ln_gamma, ln_beta, W, b):
    global last_results
    from concourse import bass_utils

    x = np.ascontiguousarray(np.asarray(x, dtype=np.float32))
    edge_attr = np.asarray(edge_attr, dtype=np.float32)
    W_f = np.ascontiguousarray(np.asarray(W, dtype=np.float32))
    b_f = np.asarray(b, dtype=np.float32)
    gamma = np.asarray(ln_gamma, dtype=np.float32)
    beta = np.asarray(ln_beta, dtype=np.float32)
    ei = np.asarray(edge_index)

    affine = not (np.all(gamma == 1.0) and np.all(beta == 0.0))

    plan = _build_plan(ei)
    EP = plan["EP"]

    key = (EP, tuple(plan["chunks"]), affine)
    if key not in _kernel_cache:
        _kernel_cache.clear()
        _kernel_cache[key] = _build_bass(EP, plan["chunks"], affine)
    nc = _kernel_cache[key]

    ea_plus_b = edge_attr + b_f[None, :]

    in_maps = []
    slots = []
    for c in range(N_CORES):
        ci = _prep_core_inputs(plan, c, ei, ea_plus_b)
        m = {
            "x": x,
            "src_idx": ci["src_idx"],
            "dst_idx": ci["dst_idx"],
            "ea": ci["ea"],
            "W": W_f,
        }
        if affine:
            m["gamma"] = gamma
            m["beta"] = beta
        in_maps.append(m)
        slots.append(ci["slot"])

    res = bass_utils.run_bass_kernel_spmd(nc, in_maps, core_ids=list(range(N_CORES)))
    last_results = res

    out = np.empty((N_EDGES, D), dtype=np.float32)
    EPC = plan["EPC"]
    for c in range(N_CORES):
        oc = res.results[c]["out"].T  # [EP, D]
        sl = slots[c]
        valid = sl >= 0
        out[c * EPC + sl[valid]] = oc[valid]
    return out
